# revision 19
# baseline (speedup 1.0000x reference)
"""Trainium2 Bass kernel for nn_Attention_49907519980190.

Reference computation (b=2, n=2048, dim=1024, h=16, d=64):
    q = (x @ w_q)   -> (b, h, n, d)
    k, v = split(x @ w_vk)
    dots = (q @ k^T) * sqrt(d)          # NOTE: multiplies by 8
    attn = softmax(dots)
    out = (attn @ v) reassembled -> (b, n, h*d) @ w_out

Sharding (8 cores): batch x head-group parallel. Core c handles batch
b = c // 4 and heads 4*(c % 4) .. 4*(c % 4) + 4. Column-parallel
q/k/v projections, row-parallel out projection; the host sums the four
partial outputs per batch (the "all-reduce" of row-parallel TP).

Numerics: everything pre-softmax runs in fp16 (11-bit mantissa).
fp16 x/w give the projections ~3e-4 rel err and the fp16 q/k single-pass
dots an absolute scaled-logit error ~0.04, i.e. softmax weight errors
of a few percent on the non-dominant mass only (softmax here is ~97%
one-hot and normalization cancels the dominant term's error). Total
output rel err ~3e-3 vs the 2e-2 gate.

Schedule: x^T is produced by the DMA xbar transpose (fp16) rather than
the PE; projections are single-pass fp16; the attention loop is
software-pipelined (dots -> max -> exp -> DMA-transpose -> PV) with the
row max split 3:1 between DVE and GPSIMD and exp on the scalar engine.
Attention probabilities and the post-softmax path are fp16; the softmax
denominator comes for free as a 65th all-ones column appended per-head
to V. DMA transposes all stay on the sync queue (two queues
concurrently corrupts data - measured in a previous session).
"""

import numpy as np

import concourse.bass as bass
import concourse.mybir as mybir
import concourse.tile as tile
from concourse import bacc
from concourse.bass_utils import run_bass_kernel_spmd
from concourse.masks import make_identity

F32 = mybir.dt.float32
BF16 = mybir.dt.bfloat16
FP16 = mybir.dt.float16
SUB = mybir.AluOpType.subtract
MULT = mybir.AluOpType.mult
MAX = mybir.AluOpType.max
AX = mybir.AxisListType.X
EXP = mybir.ActivationFunctionType.Exp

P = 128      # partitions
NTOK = 2048  # tokens per core (one batch slice)
DIM = 1024   # model dim
E = 256      # per-core projection width (4 heads x 64)
NH = 4       # heads per core
D = 64       # head dim
D1 = 65      # head dim + ones column (denominator trick)
KO = 8       # contraction chunks of 128 over DIM
TT = 16      # token tiles of 128
NG = 4       # token groups (of 512)
SCALE = 8.0  # sqrt(D); reference MULTIPLIES by it


def build_attention_nc():
    nc = bacc.Bacc("TRN2", target_bir_lowering=False, debug=False)

    x = nc.declare_dram_parameter("x", [NTOK, DIM], F32, isOutput=False)
    wq = nc.declare_dram_parameter("wq", [DIM, E], F32, isOutput=False)
    wk = nc.declare_dram_parameter("wk", [DIM, E], F32, isOutput=False)
    wv = nc.declare_dram_parameter("wv", [DIM, E], F32, isOutput=False)
    wo = nc.declare_dram_parameter("wo", [E, DIM], F32, isOutput=False)
    y = nc.declare_dram_parameter("y", [NTOK, DIM], F32, isOutput=True)

    with tile.TileContext(nc) as tc:
        with tc.tile_pool(name="persist", bufs=1) as persist:
            # Q^T / K^T per head-pair: partition rows 0:64 = head 2m,
            # 64:128 = head 2m+1; middle index m in {0,1}
            QTs = persist.tile([P, 2, NTOK], FP16)
            KTs = persist.tile([P, 2, NTOK], FP16)
            # x^T fp16: [dim_low, dim_block, token]
            xT16 = persist.tile([P, KO, NTOK], FP16)
            # V natural [tok_low, tok_tile, head*(64+ones)] fp16
            Vb = persist.tile([P, TT, NH * D1], FP16)
            Ob = persist.tile([P, TT, E], FP16)
            # O^T [emb_low, emb_hi(2), tok]
            OTb = persist.tile([P, 2, NTOK], FP16)
            wq16 = persist.tile([P, KO, E], FP16)
            wk16 = persist.tile([P, KO, E], FP16)
            wv16 = persist.tile([P, KO, E], FP16)
            wob = persist.tile([P, 2, DIM], FP16)
            # ones columns of Vb (written once)
            vb4 = Vb.rearrange("p t (h c) -> p t h c", c=D1)
            nc.vector.memset(vb4[:, :, :, D:D1], 1.0)

            # ---------- Phase A/B: load x + weights, cast fp16, xbar x^T
            with tc.tile_pool(name="stage", bufs=4) as stage:
                # weights first: K/Q projections are gated on these
                for wi, (wsrc, wdst) in enumerate(
                    ((wk, wk16), (wq, wq16), (wv, wv16))
                ):
                    wf = stage.tile([P, KO, E], F32, tag=f"wf{wi}", bufs=1)
                    nc.sync.dma_start(
                        out=wf,
                        in_=wsrc[:, :].rearrange("(ko p) e -> p ko e", p=P),
                    )
                    nc.vector.tensor_copy(out=wdst, in_=wf)
                wof = stage.tile([P, 2, DIM], F32, tag="wof", bufs=1)
                nc.scalar.dma_start(
                    out=wof, in_=wo[:, :].rearrange("(eo p) d -> p eo d", p=P)
                )
                nc.vector.tensor_copy(out=wob, in_=wof)
                for tt in range(TT):
                    ts = slice(tt * P, (tt + 1) * P)
                    xf = stage.tile([P, DIM], F32, tag="xf")
                    ldq = nc.scalar if tt % 2 == 0 else nc.sync
                    ldq.dma_start(out=xf, in_=x[ts, :])
                    xf16 = stage.tile([P, DIM], FP16, tag="xf16")
                    ceng = (nc.scalar, nc.vector)[tt % 2]
                    if ceng is nc.scalar:
                        ceng.copy(out=xf16, in_=xf)
                    else:
                        ceng.tensor_copy(out=xf16, in_=xf)
                    nc.sync.dma_start_transpose(out=xT16[:, :, ts], in_=xf16)

            # ---------- Phase C: single-pass fp16 projections
            with tc.tile_pool(name="psA", bufs=4, space="PSUM") as psA:
                for g in range(NG):
                    ns = slice(g * 512, (g + 1) * 512)
                    for wsrc16, dst in ((wk16, KTs), (wq16, QTs)):
                        for m in range(2):
                            ms = slice(m * P, (m + 1) * P)
                            pr = psA.tile([P, 512], F32, tag="pr")
                            for c in range(KO):
                                nc.tensor.matmul(
                                    pr[:, :], wsrc16[:, c, ms], xT16[:, c, ns],
                                    start=(c == 0), stop=(c == KO - 1),
                                )
                            nc.scalar.copy(out=dst[:, m, ns], in_=pr)
                    for lt in range(4):
                        tm = g * 4 + lt
                        tsl = slice(tm * P, (tm + 1) * P)
                        pr = psA.tile([P, 512], F32, tag="pr")
                        for c in range(KO):
                            nc.tensor.matmul(
                                pr[:, :E], xT16[:, c, tsl], wv16[:, c, :],
                                start=(c == 0), stop=(c == KO - 1),
                            )
                        nc.vector.tensor_copy(
                            out=vb4[:, tm, :, 0:D],
                            in_=pr[:, :E].rearrange("p (h c) -> p h c", c=D),
                        )

            # ---------- Phase D: attention + fused output projection,
            # it-outer / h-inner so each token block's O^T + y-proj issues as
            # soon as its 4 heads' PV are done (no serial epilogue phase)
            with (
                tc.tile_pool(name="psS", bufs=7, space="PSUM") as psS,
                tc.tile_pool(name="psO", bufs=1, space="PSUM") as psO,
                tc.tile_pool(name="attn_sb", bufs=3) as attn_sb,
                tc.tile_pool(name="attn_small", bufs=12) as attn_small,
            ):
                pending = []

                def issue_dots(h, it):
                    isl = slice(it * P, (it + 1) * P)
                    hr = slice((h % 2) * D, (h % 2) * D + D)
                    hp = h // 2
                    quarters = []
                    for nn in range(4):
                        Sq = psS.tile([P, 512], F32, tag="S")
                        quarters.append(Sq)
                    for nn in range(4):
                        ns = slice(nn * 512, (nn + 1) * 512)
                        nc.tensor.matmul(
                            quarters[nn][:, :],
                            QTs[hr, hp, isl], KTs[hr, hp, ns],
                            start=True, stop=True,
                        )
                    return quarters

                def issue_softmax(h, it, quarters):
                    mx4 = attn_small.tile([P, 4], F32, tag="mx4")
                    for nn in range(4):
                        nc.vector.tensor_reduce(
                            out=mx4[:, nn:nn + 1], in_=quarters[nn], axis=AX, op=MAX
                        )
                    nmx = attn_small.tile([P, 1], F32, tag="nmx")
                    nc.vector.tensor_reduce(
                        out=nmx, in_=mx4, axis=AX, op=MAX, negate=True
                    )
                    bias8 = attn_small.tile([P, 1], F32, tag="bias8")
                    nc.vector.tensor_scalar_mul(bias8, nmx, SCALE)
                    u = attn_sb.tile([P, NTOK], FP16, tag="u", bufs=8)
                    for nn in range(4):
                        cs = slice(nn * 512, (nn + 1) * 512)
                        nc.scalar.activation(
                            out=u[:, cs], in_=quarters[nn], func=EXP,
                            bias=bias8, scale=SCALE,
                        )
                    PT = attn_sb.tile([P, TT, P], FP16, tag="PT", bufs=10)
                    nc.sync.dma_start_transpose(out=PT, in_=u)
                    return PT

                def issue_pv_batch(batch):
                    # up to 2 heads share one PSUM bank; one strided
                    # reciprocal covers both denominators
                    O_ps = psO.tile([P, 2, D1], F32, tag="O")
                    for k, (h, it, PT) in enumerate(batch):
                        for jo in range(TT):
                            nc.tensor.matmul(
                                O_ps[:, k, :], PT[:, jo, :],
                                Vb[:, jo, h * D1:(h + 1) * D1],
                                start=(jo == 0), stop=(jo == TT - 1),
                            )
                    rec = attn_small.tile([P, 2], F32, tag="rec")
                    nc.vector.reciprocal(
                        out=rec[:, :len(batch)], in_=O_ps[:, :len(batch), D:D1]
                    )
                    for k, (h, it, PT) in enumerate(batch):
                        nc.scalar.activation(
                            out=Ob[:, it, h * D:(h + 1) * D], in_=O_ps[:, k, :D],
                            func=mybir.ActivationFunctionType.Copy,
                            scale=rec[:, k:k + 1],
                        )

                step = 0
                for it in range(TT):
                    for h in range(NH):
                        quarters = issue_dots(h, it)
                        # batch PV issue in pairs every other iteration so the
                        # dots blocks form long dense PE-array stretches
                        if step % 2 == 1 and len(pending) > 5:
                            issue_pv_batch([pending.pop(0), pending.pop(0)])
                        PT = issue_softmax(h, it, quarters)
                        pending.append((h, it, PT))
                        step += 1
                while pending:
                    issue_pv_batch([pending.pop(0) for _ in range(min(2, len(pending)))])

            # ---------- Phase E: O^T (DMA xbar) then y = O @ wo
            with (
                tc.tile_pool(name="psE", bufs=3, space="PSUM") as psE,
                tc.tile_pool(name="ysb", bufs=3) as ysb,
            ):
                for it in range(TT):
                    nc.sync.dma_start_transpose(
                        out=OTb[:, :, it * P:(it + 1) * P], in_=Ob[:, it, :]
                    )
                for it in range(TT):
                    ts = slice(it * P, (it + 1) * P)
                    for n in range(2):
                        ns = slice(n * 512, (n + 1) * 512)
                        yp = psE.tile([P, 512], F32, tag="yp")
                        for eo in range(2):
                            nc.tensor.matmul(
                                yp[:, :], OTb[:, eo, ts], wob[:, eo, ns],
                                start=(eo == 0), stop=(eo == 1),
                            )
                        yo = ysb.tile([P, 512], F32, tag="yo")
                        eng = nc.scalar if n == 0 else nc.vector
                        if eng is nc.scalar:
                            eng.copy(out=yo, in_=yp)
                        else:
                            eng.tensor_copy(out=yo, in_=yp)
                        dq = nc.sync if n == 0 else nc.scalar
                        dq.dma_start(out=y[ts, ns], in_=yo)

    nc.compile()
    return nc


_NC_CACHE = None


def _get_nc():
    global _NC_CACHE
    if _NC_CACHE is None:
        _NC_CACHE = build_attention_nc()
    return _NC_CACHE


def kernel(x, w_q, w_vk, w_out, **run_kwargs):
    """Full inputs in, full output out. Shards over 8 NeuronCores."""
    b, n, dim = x.shape
    assert (b, n, dim) == (2, 2048, 1024)
    w_k = w_vk[:, :1024]
    w_v = w_vk[:, 1024:]

    in_maps = []
    for c in range(8):
        bi = c // 4
        hg = c % 4
        cs = slice(hg * E, (hg + 1) * E)
        in_maps.append({
            "x": np.ascontiguousarray(x[bi]).astype(np.float32),
            "wq": np.ascontiguousarray(w_q[:, cs]).astype(np.float32),
            "wk": np.ascontiguousarray(w_k[:, cs]).astype(np.float32),
            "wv": np.ascontiguousarray(w_v[:, cs]).astype(np.float32),
            "wo": np.ascontiguousarray(w_out[cs, :]).astype(np.float32),
        })

    nc = _get_nc()
    res = run_bass_kernel_spmd(nc, in_maps, core_ids=list(range(8)), **run_kwargs)
    out = np.zeros((2, 2048, 1024), dtype=np.float32)
    for c in range(8):
        out[c // 4] += res.results[c]["y"]
    if run_kwargs:
        kernel.last_results = res
    return out
